# revision 49
# baseline (speedup 1.0000x reference)
"""Multi-head cross-attention (MHAForCrossFusion) on 8 Trainium2 cores.

Sharding: 2-way data-parallel over batch x 4-way tensor-parallel over heads.
Core ci owns batch b = ci // 4 and head-group hg = ci % 4 (4 of 16 heads,
CW = 256 features: columns of Wq/Wk/Wv, rows of Wo). Each core emits a
[2048, 1024] partial of its batch's output projection; the host sums the
4 head-group partials per batch (+ bo).

Per-core device program (all matmuls fp16, PSUM accumulation fp32):
 - q/k/v arrive fp16; DMA-xbar transposing loads produce feature-major
   xT tiles directly (no PE transposes), triggers split over both HWDGE
   engines, loads ordered k -> v -> q by first use
 - qm/km feature-major [128, g, t] via W.T-tiled matmuls + DVE bias-add
 - vm token-major [t, hv] via xT_v-stationary matmuls, with a ones-column
   appended per head (softmax denominator trick); bias via a K=1 ones-row
   matmul
 - scores S.T = km.T @ qm per head (K=64, two heads row-packed in the PE),
   2-bank PSUM groups double-buffered; exp via one ACT instr per group
   (N=1024, 1/sqrt(hd) scale folded in), output fp16
 - ctx_aug[0:65] = [vm | 1].T @ expS accumulated over key tiles:
   rows 0:64 unnormalized context, row 64 = softmax denominator
 - normalize (deferred one pass so the 3.4us DVE reciprocal hides under
   the next pass's matmuls; a data-dependency "fence" pins the broadcast
   matmul behind pt-9/12 so Tile cannot hoist it): DVE reciprocal scaled
   by RC_SCALE into fp16, K=1 ones matmul broadcast, DVE multiply ->
   ctxn fp16 (host divides the summed partials by RC_SCALE)
 - out-projection at the end: out[t, :] += ctxn.T @ Wo_slice.T over both
   feature groups, ACT copy to fp16, DMA out; the final pass's
   reciprocals overlap the earlier chunks' out-projections
"""

import os

import numpy as np

import concourse.bass as bass
import concourse.mybir as mybir
import concourse.tile as tile
from concourse import bass_utils
from concourse.masks import make_identity

N_CORES = 8
B, L, D = 2, 2048, 1024
NH, HD = 16, 64
MH, MT = 4, 2  # head-group ways x batch ways
CW = (NH // MH) * HD  # 256 features per core (4 heads)
SCALE = 1.0 / np.sqrt(HD)

F32 = mybir.dt.float32
F16 = mybir.dt.float16
RC_SCALE = 4096.0  # keeps 1/denom in fp16 normal range; host divides back

# Schraudolph-style fast exp, fp16-bit variant: fp16bits(exp(s/8)) ~=
# round(EXPA*s + EXPB). Max rel err ~3% on |s/8|<=6; used on a subset of
# key-tiles to offload the Scalar engine (the softmax bottleneck) onto DVE.
EXPA = float(0.125 * 1024 * np.log2(np.e))
EXPB = 15360.0 - 45.0
DVE_EXP_PTS = ()  # disabled: DVE is busier than ACT; keep exp on ACT
I16 = mybir.dt.int16


def _split_matmul_waits(nc):
    """fp32/fp32r matmuls lower to a self-loading LDW whose ISA struct has a
    single sem-wait slot (HWDGE DMA likewise); walrus rejects >1 wait. Move
    extra waits onto same-engine NoOps inserted right before the matmul
    (program order on the sequencer preserves the happens-before)."""
    for f in nc.m.functions:
        for bb in f.blocks:
            insts = list(bb.instructions)
            out = []
            for inst in insts:
                si = inst.sync_info
                if si is not None and len(si.on_wait) > 1:
                    for w in si.on_wait[:-1]:
                        nop = mybir.InstNoOp(
                            name=nc.get_next_instruction_name(),
                            ins=[],
                            outs=[],
                            engine=inst.engine,
                            bass_nofuse=True,
                        )
                        nop.sync_info = mybir.SyncInfo(on_wait=[w], on_update=[])
                        out.append(nop)
                    inst.sync_info = mybir.SyncInfo(
                        on_wait=[si.on_wait[-1]], on_update=si.on_update
                    )
                out.append(inst)
            if len(out) != len(insts):
                bb.instructions = out
    return nc


DC_ = D // 128


def build_nc():
    nc = bass.Bass("TRN2", target_bir_lowering=False, debug=False)

    xq = nc.dram_tensor("xq", [L, D], F16, kind="ExternalInput").ap()
    xk = nc.dram_tensor("xk", [L, D], F16, kind="ExternalInput").ap()
    xv = nc.dram_tensor("xv", [L, D], F16, kind="ExternalInput").ap()
    # weights arrive pre-packed on the host into partition-major layout so
    # each load is one contiguous descriptor per partition (a [D, CW] slice
    # via rearrange costs ~1k strided 512B descriptors on the HWDGE)
    wqt = nc.dram_tensor("wqt", [128, DC_ * CW], F16, kind="ExternalInput").ap()
    wkt = nc.dram_tensor("wkt", [128, DC_ * CW], F16, kind="ExternalInput").ap()
    wvt = nc.dram_tensor("wvt", [128, DC_ * CW], F16, kind="ExternalInput").ap()
    wot = nc.dram_tensor("wot", [128, 2 * D], F16, kind="ExternalInput").ap()
    bq = nc.dram_tensor("bq", [128, 2], F32, kind="ExternalInput").ap()
    bk = nc.dram_tensor("bk", [128, 2], F32, kind="ExternalInput").ap()
    bv = nc.dram_tensor("bv", [1, CW], F16, kind="ExternalInput").ap()
    out_p = nc.dram_tensor("out_p", [L, D], F16, kind="ExternalOutput").ap()

    DC = D // 128  # 8 contraction tiles for the projections
    NT = L // 128  # 16 token tiles per core
    NCH = L // 512  # 4 query chunks
    with tile.TileContext(nc) as tc:
        with (
            tc.tile_pool(name="singles", bufs=1) as singles,
            tc.tile_pool(name="acts", bufs=1) as acts,
            tc.tile_pool(name="small", bufs=3) as small,
            tc.tile_pool(name="esp", bufs=3) as esp,
            tc.tile_pool(name="psum", bufs=1, space="PSUM") as pp,
        ):
            ones_col = singles.tile([1, 128], F16)
            nc.vector.memset(ones_col, 1.0)
            ones64 = singles.tile([1, 64], F16)
            nc.vector.memset(ones64, 1.0)

            # transposing loads xT_*[p, c, t] = x[t, c*128 + p], interleaved
            # with the weight loads in consumption order (k first, q/wo last)
            xT = {}
            w_sb = {}
            b_sb = {}

            def load_xt(name, dram):
                # alternate trigger engines: HWDGE triggers cost ~1.3us each
                # on their issuing queue, so split across both hwdge engines
                xT[name] = acts.tile([128, DC, L], F16, name=f"{name}T")
                for th in range(2):
                    ts_ = slice(th * (L // 2), (th + 1) * (L // 2))
                    for c in range(DC):
                        nc.sync.dma_start_transpose(
                            xT[name][:, c, ts_], dram[ts_, c * 128 : (c + 1) * 128]
                        )

            def load_w(name, dram):
                w = singles.tile([128, DC, CW], F16, name=name + "_sb")
                nc.sync.dma_start(w, dram.rearrange("p (c h) -> p c h", h=CW))
                w_sb[name] = w

            def load_b(name, dram, shape, dt_):
                b = singles.tile(shape, dt_, name=name + "_sb")
                nc.sync.dma_start(b, dram)
                b_sb[name] = b

            load_w("wk", wkt)
            load_b("bk", bk, [128, 2], F32)
            load_xt("k", xk)
            load_w("wv", wvt)
            load_b("bv", bv, [1, CW], F16)
            load_xt("v", xv)
            load_w("wq", wqt)
            load_b("bq", bq, [128, 2], F32)
            wot_sb = singles.tile([128, 2, D], F16)
            nc.sync.dma_start(wot_sb, wot.rearrange("p (g d) -> p g d", d=D))
            bq_sb, bk_sb, bv_sb = b_sb["bq"], b_sb["bk"], b_sb["bv"]

            qm = acts.tile([128, 2, L], F16)  # feature-major projections
            km = acts.tile([128, 2, L], F16)
            vma = acts.tile([128, NT, MH, 66], F16)
            ctxn = acts.tile([128, 2, L], F16)

            # ones column of the augmented V (col 64 of each head group)
            nc.vector.memset(vma[:, :, :, 64], 1.0)

            # ---- K/Q projections: feature-major [128, g, t] ----
            def proj_chunk(src, wname, bias_sb, dst, ch):
                for g in range(2):
                    ps = pp.tile([128, 512], F32, tag="a", bufs=2, name="ps")
                    for dc in range(DC):
                        nc.tensor.matmul(
                            ps,
                            lhsT=w_sb[wname][:, dc, g * 128 : (g + 1) * 128],
                            rhs=xT[src][:, dc, ch * 512 : (ch + 1) * 512],
                            start=(dc == 0),
                            stop=(dc == DC - 1),
                        )
                    nc.vector.tensor_scalar_add(
                        dst[:, g, ch * 512 : (ch + 1) * 512],
                        ps,
                        bias_sb[:, g : g + 1],
                    )

            for ch in range(NCH):
                proj_chunk("k", "wk", bk_sb, km, ch)

            # ---- V projection: token-major [t, hv] + bias ones-row ----
            for tt in range(NT):
                ps = pp.tile([128, 512], F32, tag="a", bufs=2, name="psv")
                pv = ps[:, 0:CW]
                for dc in range(DC):
                    nc.tensor.matmul(
                        pv,
                        lhsT=xT["v"][:, dc, tt * 128 : (tt + 1) * 128],
                        rhs=w_sb["wv"][:, dc, :],
                        start=(dc == 0),
                        stop=False,
                    )
                nc.tensor.matmul(
                    pv, lhsT=ones_col, rhs=bv_sb, start=False, stop=True
                )
                nc.vector.tensor_copy(
                    vma[:, tt, :, 0:64], pv.rearrange("p (h c) -> p h c", c=64)
                )

            # Q arrives via plain row loads (one cheap descriptor each) + PE
            # transposes emitted after V-proj: this takes Q's ~20us of
            # xbar-DMA descriptor generation off the serial HWDGE chain that
            # gates the start of attention. K/V keep the xbar path (their
            # descgen overlaps K/V projection compute).
            ident = singles.tile([128, 128], F16)
            make_identity(nc, ident)
            xT["q"] = acts.tile([128, DC, L], F16, name="qT")
            for tt in range(NT):
                row = esp.tile([128, D], F16, tag="xrow", bufs=3, name="xrow")
                nc.sync.dma_start(row, xq[tt * 128 : (tt + 1) * 128, :])
                tp = pp.tile([128, D], F16, tag="a", bufs=2, name="tp")
                for c in range(DC):
                    nc.tensor.transpose(
                        tp[:, c * 128 : (c + 1) * 128],
                        row[:, c * 128 : (c + 1) * 128],
                        ident,
                    )
                nc.vector.tensor_copy(
                    xT["q"][:, :, tt * 128 : (tt + 1) * 128],
                    tp.rearrange("p (c t) -> p c t", t=128),
                )

            for ch in range(NCH):
                proj_chunk("q", "wq", bq_sb, qm, ch)

            # ---- attention + out-projection, per 512-query chunk ----
            # normalize is deferred one (g-)pass and the out-projection one
            # chunk, so their latency (DVE reciprocal chain, PSUM ring
            # releases) hides under the next pass's score/ctx matmuls.
            def norm_recip(ctx, h):
                rc = small.tile([1, 512], F32, tag="rc")
                nc.vector.reciprocal(rc, ctx[h][64:65, :])
                rc16 = small.tile([1, 512], F16, tag="rc16")
                nc.vector.tensor_scalar_mul(rc16, rc, RC_SCALE)
                return rc16

            def norm_fence(e):
                # ones vector that data-depends on this pass's pt-9/12 exp:
                # pins the broadcast matmul's scheduled position so the slow
                # DVE reciprocal (3.4us, mis-modeled as cheap) finishes first
                f = small.tile([1, 64], F16, tag="fence")
                nc.vector.tensor_scalar(
                    f, e[0:1, 0, 0:64], 0.0, 1.0, mybir.AluOpType.mult,
                    mybir.AluOpType.add,
                )
                return f

            def norm_apply(ctx, g, ls, h, rc16, ones_f):
                nc.tensor.matmul(ctx[h][64:128, :], lhsT=ones_f, rhs=rc16)
                bcs = small.tile([64, 512], F32, tag="bcs")
                nc.vector.tensor_copy(bcs, ctx[h][64:128, :])
                nc.vector.tensor_mul(
                    ctxn[h * 64 : (h + 1) * 64, g, ls], ctx[h][0:64, :], bcs
                )

            def outproj_tt(ch, tt):
                # ob copy on ACT (idle in the tail) so DVE is free for the
                # final pass's reciprocals
                t0 = ch * 512 + tt * 128
                po = pp.tile([128, 2, 512], F32, tag="a", bufs=2, name="po")
                for eh in range(2):
                    for g in range(2):
                        nc.tensor.matmul(
                            po[:, eh, :],
                            lhsT=ctxn[:, g, t0 : t0 + 128],
                            rhs=wot_sb[:, g, eh * 512 : (eh + 1) * 512],
                            start=(g == 0),
                            stop=(g == 1),
                        )
                ob = small.tile([128, D], F16, tag="ob")
                nc.scalar.copy(ob, po.rearrange("p h t -> p (h t)"))
                nc.sync.dma_start(out_p[t0 : t0 + 128, :], ob)

            pending_norm = None  # (ctx, g, ls) of the previous pass
            for ch in range(NCH):
                ls = slice(ch * 512, (ch + 1) * 512)
                for g in range(2):
                    ctx = [
                        pp.tile([128, 512], F32, tag="b", bufs=4, name=f"ctx{h}")
                        for h in range(2)
                    ]
                    for pt in range(NT):
                        ks = slice(pt * 128, (pt + 1) * 128)
                        sc = pp.tile([128, 2, 512], F32, tag="a", bufs=2, name="sc")
                        for h in range(2):
                            hs = slice(h * 64, (h + 1) * 64)
                            nc.tensor.matmul(
                                sc[:, h, :],
                                lhsT=km[hs, g, ks],
                                rhs=qm[hs, g, ls],
                                tile_position=(h * 64, 0),
                            )
                        es = esp.tile([128, 2, 512], F16, tag="es", name="es")
                        if pt in DVE_EXP_PTS:
                            nc.vector.tensor_scalar(
                                es.rearrange("p h t -> p (h t)").bitcast(I16),
                                sc.rearrange("p h t -> p (h t)"),
                                EXPA,
                                EXPB,
                                mybir.AluOpType.mult,
                                mybir.AluOpType.add,
                            )
                        else:
                            nc.scalar.activation(
                                es.rearrange("p h t -> p (h t)"),
                                sc.rearrange("p h t -> p (h t)"),
                                mybir.ActivationFunctionType.Exp,
                                scale=SCALE,
                            )
                        for h in range(2):
                            nc.tensor.matmul(
                                ctx[h][0:65, :],
                                lhsT=vma[:, pt, g * 2 + h, 0:65],
                                rhs=es[:, h, :],
                                start=(pt == 0),
                                stop=(pt == NT - 1),
                            )
                        if pending_norm is not None:
                            pctx, pg, pls = pending_norm
                            if pt == 2:
                                rc16s = [norm_recip(pctx, h) for h in range(2)]
                            elif pt == 9:
                                norm_apply(pctx, pg, pls, 0, rc16s[0], norm_fence(es))
                            elif pt == 12:
                                norm_apply(pctx, pg, pls, 1, rc16s[1], norm_fence(es))
                                pending_norm = None
                    pending_norm = (ctx, g, ls)
            # tail: final pass's reciprocals (DVE) run under the out-projection
            # of the already-normalized chunks (PE + ACT copies)
            rc16s = [norm_recip(pending_norm[0], h) for h in range(2)]
            for ch in range(NCH - 1):
                for tt in range(4):
                    outproj_tt(ch, tt)
            for h in range(2):
                norm_apply(*pending_norm, h, rc16s[h], ones64)
            for tt in range(4):
                outproj_tt(NCH - 1, tt)
    return _split_matmul_waits(nc)


_NC_CACHE = None
LAST_RESULTS = None


def kernel(q, k, v, attention_mask, Wq, bq, Wk, bk, Wv, bv, Wo, bo):
    global _NC_CACHE, LAST_RESULTS
    assert np.asarray(attention_mask).all(), "kernel assumes all-ones mask"
    if _NC_CACHE is None:
        _NC_CACHE = build_nc()
    nc = _NC_CACHE

    c = np.ascontiguousarray
    qb = np.asarray(q, np.float32).astype(np.float16)
    kb = np.asarray(k, np.float32).astype(np.float16)
    vb = np.asarray(v, np.float32).astype(np.float16)
    wqt = np.asarray(Wq, np.float32).T.astype(np.float16)
    wkt = np.asarray(Wk, np.float32).T.astype(np.float16)
    wvt = np.asarray(Wv, np.float32).T.astype(np.float16)
    wot = np.asarray(Wo, np.float32).T.astype(np.float16)

    def pack_w(a):  # [1024, 256] -> [128, 8*256], partition-major
        return np.ascontiguousarray(
            a.reshape(8, 128, CW).transpose(1, 0, 2).reshape(128, 8 * CW)
        )

    def pack_wo(a):  # [256, 1024] -> [128, 2*1024]
        return np.ascontiguousarray(
            a.reshape(2, 128, D).transpose(1, 0, 2).reshape(128, 2 * D)
        )
    bqf = np.asarray(bq, np.float32)
    bkf = np.asarray(bk, np.float32)
    bvb = np.asarray(bv, np.float32).astype(np.float16)

    in_maps = []
    for ci in range(N_CORES):
        b, hg = ci // MH, ci % MH
        hs = slice(hg * CW, (hg + 1) * CW)
        in_maps.append(
            {
                "xq": c(qb[b]),
                "xk": c(kb[b]),
                "xv": c(vb[b]),
                "wqt": pack_w(wqt[:, hs]),
                "wkt": pack_w(wkt[:, hs]),
                "wvt": pack_w(wvt[:, hs]),
                "wot": pack_wo(wot[hs, :]),
                "bq": c(bqf[hs].reshape(2, 128).T),
                "bk": c(bkf[hs].reshape(2, 128).T),
                "bv": c(bvb[hs].reshape(1, CW)),
            }
        )

    res = bass_utils.run_bass_kernel_spmd(nc, in_maps, core_ids=list(range(N_CORES)))
    LAST_RESULTS = res
    out = np.zeros((B, L, D), np.float32)
    for ci, r in enumerate(res.results):
        out[ci // MH] += r["out_p"].astype(np.float32)
    out *= 1.0 / RC_SCALE
    out += np.asarray(bo, np.float32)[None, None, :]
    return out
